# revision 16
# baseline (speedup 1.0000x reference)
"""Multi-head QKV tanh-attention (B=2,S=2048,D=1024,H=16) on 8 TRN2 cores.

Sharding: core = (batch, head_group) on a 2x4 grid. Each core computes the
output column slice out[b, :, hg*256:(hg+1)*256] for its 4 heads; no
cross-core communication (host gathers slices). All transposes are done on
the host: the device receives X^T, column-sliced weights, and
(residual+bo)^T, and produces out^T for its slice.

Per-core device program (all matmuls in float32r at full PE rate):
  qT/kT = W^T X^T (+bias per-partition), v = X W_v (+bias bcast)
  per head, per 128-row key block:
    scoresT = kT_blk^T qT          (PSUM)
    attT    = tanh(scoresT/scale)  (ScalarE, scale folded into activation)
    outT   += v_blk^T attT         (PSUM accumulation over key blocks)
  outT + (residual+bo)^T -> DMA out
"""
import sys

if "/opt/trn_rl_repo" not in sys.path:
    sys.path.insert(0, "/opt/trn_rl_repo")

import numpy as np

B, S, D, H = 2, 2048, 1024, 16
N_CORES = 8
HG = 4                # head groups = cores per batch
HPC = H // HG         # heads per core (4)
HD = D // H           # head dim (64)
DQ = HPC * HD         # per-core output columns (256)
NB = S // 128         # key blocks (16)
KD = D // 128         # contraction chunks (8)
SCALE_INV = float(1.0 / np.sqrt(S / H))

TRACE = False
LAST_RESULT = None

_nc_cache = []


def _enable_ldw_opt():
    """Walrus elides back-to-back LDWEIGHTS with identical APs when
    --enable-ldw-opt=true; concourse hardcodes false. Consecutive matmuls
    here reuse the same stationary operand, so flip the flag."""
    from concourse import bass_utils as bu

    if getattr(bu, "_ldw_opt_patched", False):
        return
    orig = bu.run_command

    def patched(argv, **kwargs):
        argv = [
            a.replace("--enable-ldw-opt=false", "--enable-ldw-opt=true")
            if isinstance(a, str)
            else a
            for a in argv
        ]
        return orig(argv, **kwargs)

    bu.run_command = patched
    bu._ldw_opt_patched = True


def _build_nc():
    from concourse import bacc, mybir
    import concourse.tile as tile


    f32 = mybir.dt.float32
    bf16 = mybir.dt.bfloat16
    Tanh = mybir.ActivationFunctionType.Tanh

    nc = bacc.Bacc(
        "TRN2",
        target_bir_lowering=False,
        debug=False,
        enable_asserts=False,
        num_devices=N_CORES,
    )
    xt_d = nc.dram_tensor("xt", [D, S], bf16, kind="ExternalInput")
    wq_d = nc.dram_tensor("wq", [D, DQ], bf16, kind="ExternalInput")
    wk_d = nc.dram_tensor("wk", [D, DQ], bf16, kind="ExternalInput")
    wv_d = nc.dram_tensor("wv", [D, DQ], bf16, kind="ExternalInput")
    bqk_d = nc.dram_tensor("bqk", [128, 4], f32, kind="ExternalInput")
    bvb_d = nc.dram_tensor("bvb", [128, DQ], f32, kind="ExternalInput")
    rt_d = nc.dram_tensor("rt", [DQ, S], f32, kind="ExternalInput")
    ot_d = nc.dram_tensor("ot", [DQ, S], f32, kind="ExternalOutput")

    with tile.TileContext(nc) as tc:
        with (
            tc.tile_pool(name="xt", bufs=1) as xt_pool,
            tc.tile_pool(name="w", bufs=3) as w_pool,
            tc.tile_pool(name="qk", bufs=4) as qk_pool,
            tc.tile_pool(name="v", bufs=NB) as v_pool,
            tc.tile_pool(name="rt", bufs=HPC) as rt_pool,
            tc.tile_pool(name="bias", bufs=3) as bias_pool,
            tc.tile_pool(name="att", bufs=6) as att_pool,
            tc.tile_pool(name="osb", bufs=2) as out_pool,
            tc.tile_pool(name="junk", bufs=2) as junk_pool,
            # PSUM: psA [128,1024]x3 = 6 banks, pso [64,1024]x1 = 2 banks
            tc.tile_pool(name="psA", bufs=3, space="PSUM") as psA_pool,
            tc.tile_pool(name="pso", bufs=1, space="PSUM") as pso_pool,
        ):
            # ---- PE warmup: junk matmuls while input DMAs stream ------------
            jw = junk_pool.tile([128, 512], bf16, tag="junk", name="jw")
            nc.vector.memset(jw[:], 0.0)
            jp = psA_pool.tile([128, 1024], f32, tag="psA", name="jp")
            for _ in range(24):
                nc.tensor.matmul(jp[:, 0:512], jw[:, 0:128], jw[:], start=True, stop=True)

            # ---- input DMAs: few large transfers, spread over both HWDGE
            # rings (sync + scalar) and SWDGE (gpsimd) to avoid the ~0.65us
            # per-dma issue cost serializing on one queue. ------------------
            xt_all = xt_pool.tile([128, KD * S], bf16, tag="xt", name="xt_all")
            xt_r = xt_d.ap().rearrange("(c p) s -> p c s", p=128)
            nc.sync.dma_start(xt_all[:, 0 : 4 * S], xt_r[:, 0:4, :])
            nc.sync.dma_start(xt_all[:, 4 * S : 8 * S], xt_r[:, 4:8, :])
            w_tiles = {}
            for nm, dram in (("wk", wk_d), ("wq", wq_d), ("wv", wv_d)):
                t = w_pool.tile([128, KD * DQ], bf16, tag="w", name=f"{nm}_all")
                nc.scalar.dma_start(t[:], dram.ap().rearrange("(c p) q -> p c q", p=128))
                w_tiles[nm] = t
            bqk_sb = bias_pool.tile([128, 4], f32, tag="b", name="bqks")
            nc.scalar.dma_start(bqk_sb[:], bqk_d.ap())
            bq_sb = bqk_sb[:, 0:2]
            bk_sb = bqk_sb[:, 2:4]
            bvb_sb = bias_pool.tile([128, DQ], f32, tag="bvb", name="bvbs")
            nc.scalar.dma_start(bvb_sb[:], bvb_d.ap())
            rt_sb = []
            for h in range(HPC):
                t = rt_pool.tile([64, S], f32, tag="rt", name=f"rt{h}")
                nc.gpsimd.dma_start(t[:], rt_d.ap()[h * 64 : (h + 1) * 64, :])
                rt_sb.append(t)

            class _View:
                def __init__(self, base, stride):
                    self.base = base
                    self.stride = stride

                def __getitem__(self, i):
                    class _Sl:
                        def __init__(so, base, off):
                            so.base = base
                            so.off = off

                        def __getitem__(so, key):
                            rows, cols = key
                            return so.base[rows, so.off + cols.start : so.off + cols.stop]

                    return _Sl(self.base, i * self.stride)

            xt_sb = _View(xt_all, S)
            wk_sb = _View(w_tiles["wk"], DQ)
            wq_sb = _View(w_tiles["wq"], DQ)
            wv_sb = _View(w_tiles["wv"], DQ)

            qT_sb = [qk_pool.tile([128, S], bf16, tag="qk", name=f"qT{g}") for g in range(2)]
            kT_sb = [qk_pool.tile([128, S], bf16, tag="qk", name=f"kT{g}") for g in range(2)]
            v_sb = [v_pool.tile([128, DQ], bf16, tag="v", name=f"v{sb}") for sb in range(NB)]

            # ---- emission helpers -------------------------------------------
            # Generators yield after small PE chunks so attention units can be
            # interleaved between them (keeps ScalarE fed with tanh work).
            def gen_proj_qk(w_sb, b_sb, dst, g, half):
                hsl = slice(half * 1024, (half + 1) * 1024)
                ps = psA_pool.tile([128, 1024], f32, tag="psA", name="psp")
                for din in range(KD):
                    for ns in range(2):
                        nc.tensor.matmul(
                            ps[:, ns * 512 : (ns + 1) * 512],
                            w_sb[din][:, g * 128 : (g + 1) * 128],
                            xt_sb[din][:, half * 1024 + ns * 512 :
                                       half * 1024 + (ns + 1) * 512],
                            start=(din == 0),
                            stop=(din == KD - 1),
                            skip_group_check=True,
                        )
                    if din < KD - 1:
                        yield
                nc.vector.tensor_scalar_add(dst[g][:, hsl], ps[:], b_sb[:, g : g + 1])

            def gen_v(sbg):
                ps = psA_pool.tile([128, 1024], f32, tag="psA", name="psv")
                for sbi in range(4):
                    sb = sbg * 4 + sbi
                    for din in range(KD):
                        nc.tensor.matmul(
                            ps[:, sbi * DQ : (sbi + 1) * DQ],
                            xt_sb[din][:, sb * 128 : (sb + 1) * 128],
                            wv_sb[din][:, 0:DQ],
                            start=(din == 0),
                            stop=(din == KD - 1),
                            skip_group_check=True,
                        )
                    nc.vector.tensor_add(
                        v_sb[sb][:], ps[:, sbi * DQ : (sbi + 1) * DQ], bvb_sb[:]
                    )
                    if sbi < 3:
                        yield

            def run_gen(gen):
                for _ in gen:
                    pass

            o_tiles = {}
            pso_tiles = {}

            def emit_unit(g, hh, half, sb):
                # one attention unit: scoresT -> tanh -> AV for one key block
                h = 2 * g + hh
                po = hh * 64
                key = (h, half)
                if key not in pso_tiles:
                    pso_tiles[key] = pso_pool.tile([64, 1024], f32, tag="pso", name="ps_o")
                ps_o = pso_tiles[key]
                ps_s = psA_pool.tile([128, 1024], f32, tag="psA", name="pss")
                for ns in range(2):
                    nc.tensor.matmul(
                        ps_s[:, ns * 512 : (ns + 1) * 512],
                        kT_sb[g][po : po + 64, sb * 128 : (sb + 1) * 128],
                        qT_sb[g][po : po + 64, half * 1024 + ns * 512 :
                                 half * 1024 + (ns + 1) * 512],
                        start=True,
                        stop=True,
                        skip_group_check=True,
                    )
                att = att_pool.tile([128, 1024], bf16, tag="att", name="att")
                nc.scalar.activation(att[:], ps_s[:], Tanh, scale=SCALE_INV)
                for ns in range(2):
                    nc.tensor.matmul(
                        ps_o[:, ns * 512 : (ns + 1) * 512],
                        v_sb[sb][:, h * HD : (h + 1) * HD],
                        att[:, ns * 512 : (ns + 1) * 512],
                        start=(sb == 0),
                        stop=(sb == NB - 1),
                        skip_group_check=True,
                    )
                if sb == NB - 1:
                    if h not in o_tiles:
                        o_tiles[h] = out_pool.tile([64, S], f32, tag="osb", name=f"o{h}")
                    o_sb = o_tiles[h]
                    hsl = slice(half * 1024, (half + 1) * 1024)
                    nc.vector.tensor_add(o_sb[:, hsl], ps_o[:], rt_sb[h][:, hsl])
                    nc.sync.dma_start(
                        ot_d.ap()[h * 64 : (h + 1) * 64,
                                  half * 1024 : (half + 1) * 1024],
                        o_sb[:, hsl],
                    )
                    del pso_tiles[key]
                    if half == 1:
                        del o_tiles[h]

            # ---- schedule ---------------------------------------------------
            # Minimal prefix before attention can start: kT/qT of group 0,
            # s-half 0 (covers key blocks 0-7 for heads 0/1).
            run_gen(gen_proj_qk(wk_sb, bk_sb, kT_sb, 0, 0))
            run_gen(gen_proj_qk(wq_sb, bq_sb, qT_sb, 0, 0))

            # Fillers, ordered by when their results are first needed:
            #   v[sb] before unit (h0, half0, sb); kg0h1 before sb>=8 of h0;
            #   qg0h1 before h0 half1; g1 projections before heads 2/3.
            fillers = []
            for sbg in range(4):
                fillers.append(gen_v(sbg))            # 4 yields each -> 16 chunks
            fillers.append(gen_proj_qk(wk_sb, bk_sb, kT_sb, 0, 1))
            fillers.append(gen_proj_qk(wq_sb, bq_sb, qT_sb, 0, 1))
            fillers.append(gen_proj_qk(wk_sb, bk_sb, kT_sb, 1, 0))
            fillers.append(gen_proj_qk(wq_sb, bq_sb, qT_sb, 1, 0))
            fillers.append(gen_proj_qk(wk_sb, bk_sb, kT_sb, 1, 1))
            fillers.append(gen_proj_qk(wq_sb, bq_sb, qT_sb, 1, 1))

            units = []
            for h in range(HPC):
                g, hh = divmod(h, 2)
                for half in range(2):
                    for sb in range(NB):
                        units.append((g, hh, half, sb))

            # Pacing: each unit's producers are emitted before it (v[sb]
            # just-in-time; kg0h1 drained before sb=8; qg0h1 before half 1;
            # g1 projections spread over h0-half1/h1 and drained before h2).
            vgens, projgens = fillers[:4], fillers[4:]
            kg0h1, qg0h1 = projgens[0], projgens[1]
            rest = projgens[2:]

            def step(gen):
                try:
                    next(gen)
                    return True
                except StopIteration:
                    return False

            def drain(gen):
                for _ in gen:
                    pass

            pending = list(rest)
            for i, (g, hh, half, sb) in enumerate(units):
                if i < 16:
                    step(vgens[sb // 4])
                    if sb == 8:
                        drain(kg0h1)
                    elif sb < 8:
                        step(kg0h1)
                elif i == 16:
                    drain(qg0h1)
                elif i == 64:
                    for gn in pending:
                        drain(gn)
                    pending = []
                elif pending:
                    if not step(pending[0]):
                        pending.pop(0)
                emit_unit(g, hh, half, sb)
            for gn in pending:
                drain(gn)
            drain(kg0h1)
            drain(qg0h1)

    nc.compile()
    return nc


def _get_nc():
    if not _nc_cache:
        _nc_cache.append(_build_nc())
    return _nc_cache[0]


def _shard_inputs(X, residual_score, wq, wk, wv, bq, bk, bv, bo):
    import ml_dtypes

    bf = ml_dtypes.bfloat16
    xts = [np.ascontiguousarray(X[b].T).astype(bf) for b in range(B)]
    in_maps = []
    for core in range(N_CORES):
        b, hg = divmod(core, HG)
        c0 = hg * DQ
        cs = slice(c0, c0 + DQ)
        in_maps.append(
            {
                "xt": xts[b],
                "wq": np.ascontiguousarray(wq[:, cs]).astype(bf),
                "wk": np.ascontiguousarray(wk[:, cs]).astype(bf),
                "wv": np.ascontiguousarray(wv[:, cs]).astype(bf),
                "bqk": np.ascontiguousarray(
                    np.concatenate(
                        [bq[cs].reshape(2, 128).T, bk[cs].reshape(2, 128).T], axis=1
                    )
                ),
                "bvb": np.ascontiguousarray(np.broadcast_to(bv[cs], (128, DQ))),
                "rt": np.ascontiguousarray((residual_score[b][:, cs] + bo[cs]).T),
            }
        )
    return in_maps


def kernel(X, residual_score, wq, wk, wv, bq, bk, bv, bo, head_num):
    global LAST_RESULT
    from concourse.bass_utils import run_bass_kernel_spmd

    assert int(head_num) == H
    X = np.asarray(X, dtype=np.float32)
    residual_score = np.asarray(residual_score, dtype=np.float32)
    wq = np.asarray(wq, dtype=np.float32)
    wk = np.asarray(wk, dtype=np.float32)
    wv = np.asarray(wv, dtype=np.float32)
    bq = np.asarray(bq, dtype=np.float32)
    bk = np.asarray(bk, dtype=np.float32)
    bv = np.asarray(bv, dtype=np.float32)
    bo = np.asarray(bo, dtype=np.float32)

    nc = _get_nc()
    in_maps = _shard_inputs(X, residual_score, wq, wk, wv, bq, bk, bv, bo)
    res = run_bass_kernel_spmd(
        nc, in_maps, core_ids=list(range(N_CORES)), trace=TRACE
    )
    LAST_RESULT = res

    out = np.empty((B, S, D), dtype=np.float32)
    for core in range(N_CORES):
        b, hg = divmod(core, HG)
        c0 = hg * DQ
        out[b][:, c0 : c0 + DQ] = res.results[core]["ot"].T
    return (out, out)


# revision 17
# speedup vs baseline: 1.0329x; 1.0329x over previous
"""Multi-head QKV tanh-attention (B=2,S=2048,D=1024,H=16) on 8 TRN2 cores.

Sharding: core = (batch, head_group) on a 2x4 grid. Each core computes the
output column slice out[b, :, hg*256:(hg+1)*256] for its 4 heads; no
cross-core communication (host gathers slices). All transposes are done on
the host: the device receives X^T, column-sliced weights, and
(residual+bo)^T, and produces out^T for its slice.

Per-core device program (all matmuls in float32r at full PE rate):
  qT/kT = W^T X^T (+bias per-partition), v = X W_v (+bias bcast)
  per head, per 128-row key block:
    scoresT = kT_blk^T qT          (PSUM)
    attT    = tanh(scoresT/scale)  (ScalarE, scale folded into activation)
    outT   += v_blk^T attT         (PSUM accumulation over key blocks)
  outT + (residual+bo)^T -> DMA out
"""
import sys

if "/opt/trn_rl_repo" not in sys.path:
    sys.path.insert(0, "/opt/trn_rl_repo")

import numpy as np

B, S, D, H = 2, 2048, 1024, 16
N_CORES = 8
HG = 4                # head groups = cores per batch
HPC = H // HG         # heads per core (4)
HD = D // H           # head dim (64)
DQ = HPC * HD         # per-core output columns (256)
NB = S // 128         # key blocks (16)
KD = D // 128         # contraction chunks (8)
SCALE_INV = float(1.0 / np.sqrt(S / H))

TRACE = False
LAST_RESULT = None

_nc_cache = []


def _enable_ldw_opt():
    """Walrus elides back-to-back LDWEIGHTS with identical APs when
    --enable-ldw-opt=true; concourse hardcodes false. Consecutive matmuls
    here reuse the same stationary operand, so flip the flag."""
    from concourse import bass_utils as bu

    if getattr(bu, "_ldw_opt_patched", False):
        return
    orig = bu.run_command

    def patched(argv, **kwargs):
        argv = [
            a.replace("--enable-ldw-opt=false", "--enable-ldw-opt=true")
            if isinstance(a, str)
            else a
            for a in argv
        ]
        return orig(argv, **kwargs)

    bu.run_command = patched
    bu._ldw_opt_patched = True


def _build_nc():
    from concourse import bacc, mybir
    import concourse.tile as tile


    f32 = mybir.dt.float32
    bf16 = mybir.dt.bfloat16
    Tanh = mybir.ActivationFunctionType.Tanh

    nc = bacc.Bacc(
        "TRN2",
        target_bir_lowering=False,
        debug=False,
        enable_asserts=False,
        num_devices=N_CORES,
    )
    xt_d = nc.dram_tensor("xt", [128, KD * S], bf16, kind="ExternalInput")
    wq_d = nc.dram_tensor("wq", [128, KD * DQ], bf16, kind="ExternalInput")
    wk_d = nc.dram_tensor("wk", [128, KD * DQ], bf16, kind="ExternalInput")
    wv_d = nc.dram_tensor("wv", [128, KD * DQ], bf16, kind="ExternalInput")
    bqk_d = nc.dram_tensor("bqk", [128, 4], f32, kind="ExternalInput")
    bvb_d = nc.dram_tensor("bvb", [128, DQ], f32, kind="ExternalInput")
    rt_d = nc.dram_tensor("rt", [DQ, S], f32, kind="ExternalInput")
    ot_d = nc.dram_tensor("ot", [DQ, S], f32, kind="ExternalOutput")

    with tile.TileContext(nc) as tc:
        with (
            tc.tile_pool(name="xt", bufs=1) as xt_pool,
            tc.tile_pool(name="w", bufs=3) as w_pool,
            tc.tile_pool(name="qk", bufs=4) as qk_pool,
            tc.tile_pool(name="v", bufs=NB) as v_pool,
            tc.tile_pool(name="rt", bufs=HPC) as rt_pool,
            tc.tile_pool(name="bias", bufs=3) as bias_pool,
            tc.tile_pool(name="att", bufs=6) as att_pool,
            tc.tile_pool(name="osb", bufs=2) as out_pool,
            tc.tile_pool(name="junk", bufs=2) as junk_pool,
            # PSUM: psA [128,1024]x3 = 6 banks, pso [64,1024]x1 = 2 banks
            tc.tile_pool(name="psA", bufs=3, space="PSUM") as psA_pool,
            tc.tile_pool(name="pso", bufs=1, space="PSUM") as pso_pool,
        ):
            # ---- PE warmup: junk matmuls while input DMAs stream ------------
            jw = junk_pool.tile([128, 512], bf16, tag="junk", name="jw")
            nc.vector.memset(jw[:], 0.0)
            jp = psA_pool.tile([128, 1024], f32, tag="psA", name="jp")
            for _ in range(24):
                nc.tensor.matmul(jp[:, 0:512], jw[:, 0:128], jw[:], start=True, stop=True)

            # ---- input DMAs: few large transfers, spread over both HWDGE
            # rings (sync + scalar) and SWDGE (gpsimd) to avoid the ~0.65us
            # per-dma issue cost serializing on one queue. ------------------
            xt_all = xt_pool.tile([128, KD * S], bf16, tag="xt", name="xt_all")
            nc.sync.dma_start(xt_all[:, 0 : 4 * S], xt_d.ap()[:, 0 : 4 * S])
            nc.sync.dma_start(xt_all[:, 4 * S : 8 * S], xt_d.ap()[:, 4 * S : 8 * S])
            w_tiles = {}
            for nm, dram in (("wk", wk_d), ("wq", wq_d), ("wv", wv_d)):
                t = w_pool.tile([128, KD * DQ], bf16, tag="w", name=f"{nm}_all")
                nc.scalar.dma_start(t[:], dram.ap())
                w_tiles[nm] = t
            bqk_sb = bias_pool.tile([128, 4], f32, tag="b", name="bqks")
            nc.scalar.dma_start(bqk_sb[:], bqk_d.ap())
            bq_sb = bqk_sb[:, 0:2]
            bk_sb = bqk_sb[:, 2:4]
            bvb_sb = bias_pool.tile([128, DQ], f32, tag="bvb", name="bvbs")
            nc.scalar.dma_start(bvb_sb[:], bvb_d.ap())
            rt_sb = []
            for h in range(HPC):
                t = rt_pool.tile([64, S], f32, tag="rt", name=f"rt{h}")
                nc.gpsimd.dma_start(t[:], rt_d.ap()[h * 64 : (h + 1) * 64, :])
                rt_sb.append(t)

            class _View:
                def __init__(self, base, stride):
                    self.base = base
                    self.stride = stride

                def __getitem__(self, i):
                    class _Sl:
                        def __init__(so, base, off):
                            so.base = base
                            so.off = off

                        def __getitem__(so, key):
                            rows, cols = key
                            return so.base[rows, so.off + cols.start : so.off + cols.stop]

                    return _Sl(self.base, i * self.stride)

            xt_sb = _View(xt_all, S)
            wk_sb = _View(w_tiles["wk"], DQ)
            wq_sb = _View(w_tiles["wq"], DQ)
            wv_sb = _View(w_tiles["wv"], DQ)

            qT_sb = [qk_pool.tile([128, S], bf16, tag="qk", name=f"qT{g}") for g in range(2)]
            kT_sb = [qk_pool.tile([128, S], bf16, tag="qk", name=f"kT{g}") for g in range(2)]
            v_sb = [v_pool.tile([128, DQ], bf16, tag="v", name=f"v{sb}") for sb in range(NB)]

            # ---- emission helpers -------------------------------------------
            # Generators yield after small PE chunks so attention units can be
            # interleaved between them (keeps ScalarE fed with tanh work).
            def gen_proj_qk(w_sb, b_sb, dst, g, half):
                hsl = slice(half * 1024, (half + 1) * 1024)
                ps = psA_pool.tile([128, 1024], f32, tag="psA", name="psp")
                for din in range(KD):
                    for ns in range(2):
                        nc.tensor.matmul(
                            ps[:, ns * 512 : (ns + 1) * 512],
                            w_sb[din][:, g * 128 : (g + 1) * 128],
                            xt_sb[din][:, half * 1024 + ns * 512 :
                                       half * 1024 + (ns + 1) * 512],
                            start=(din == 0),
                            stop=(din == KD - 1),
                            skip_group_check=True,
                        )
                    if din < KD - 1:
                        yield
                nc.vector.tensor_scalar_add(dst[g][:, hsl], ps[:], b_sb[:, g : g + 1])

            def gen_v(sbg):
                ps = psA_pool.tile([128, 1024], f32, tag="psA", name="psv")
                for sbi in range(4):
                    sb = sbg * 4 + sbi
                    for din in range(KD):
                        nc.tensor.matmul(
                            ps[:, sbi * DQ : (sbi + 1) * DQ],
                            xt_sb[din][:, sb * 128 : (sb + 1) * 128],
                            wv_sb[din][:, 0:DQ],
                            start=(din == 0),
                            stop=(din == KD - 1),
                            skip_group_check=True,
                        )
                    nc.vector.tensor_add(
                        v_sb[sb][:], ps[:, sbi * DQ : (sbi + 1) * DQ], bvb_sb[:]
                    )
                    if sbi < 3:
                        yield

            def run_gen(gen):
                for _ in gen:
                    pass

            o_tiles = {}
            pso_tiles = {}

            def emit_unit(g, hh, half, sb):
                # one attention unit: scoresT -> tanh -> AV for one key block
                h = 2 * g + hh
                po = hh * 64
                key = (h, half)
                if key not in pso_tiles:
                    pso_tiles[key] = pso_pool.tile([64, 1024], f32, tag="pso", name="ps_o")
                ps_o = pso_tiles[key]
                ps_s = psA_pool.tile([128, 1024], f32, tag="psA", name="pss")
                for ns in range(2):
                    nc.tensor.matmul(
                        ps_s[:, ns * 512 : (ns + 1) * 512],
                        kT_sb[g][po : po + 64, sb * 128 : (sb + 1) * 128],
                        qT_sb[g][po : po + 64, half * 1024 + ns * 512 :
                                 half * 1024 + (ns + 1) * 512],
                        start=True,
                        stop=True,
                        skip_group_check=True,
                    )
                att = att_pool.tile([128, 1024], bf16, tag="att", name="att")
                nc.scalar.activation(att[:], ps_s[:], Tanh, scale=SCALE_INV)
                for ns in range(2):
                    nc.tensor.matmul(
                        ps_o[:, ns * 512 : (ns + 1) * 512],
                        v_sb[sb][:, h * HD : (h + 1) * HD],
                        att[:, ns * 512 : (ns + 1) * 512],
                        start=(sb == 0),
                        stop=(sb == NB - 1),
                        skip_group_check=True,
                    )
                if sb == NB - 1:
                    if h not in o_tiles:
                        o_tiles[h] = out_pool.tile([64, S], f32, tag="osb", name=f"o{h}")
                    o_sb = o_tiles[h]
                    hsl = slice(half * 1024, (half + 1) * 1024)
                    nc.vector.tensor_add(o_sb[:, hsl], ps_o[:], rt_sb[h][:, hsl])
                    nc.sync.dma_start(
                        ot_d.ap()[h * 64 : (h + 1) * 64,
                                  half * 1024 : (half + 1) * 1024],
                        o_sb[:, hsl],
                    )
                    del pso_tiles[key]
                    if half == 1:
                        del o_tiles[h]

            # ---- schedule ---------------------------------------------------
            # Minimal prefix before attention can start: kT/qT of group 0,
            # s-half 0 (covers key blocks 0-7 for heads 0/1).
            run_gen(gen_proj_qk(wk_sb, bk_sb, kT_sb, 0, 0))
            run_gen(gen_proj_qk(wq_sb, bq_sb, qT_sb, 0, 0))

            # Fillers, ordered by when their results are first needed:
            #   v[sb] before unit (h0, half0, sb); kg0h1 before sb>=8 of h0;
            #   qg0h1 before h0 half1; g1 projections before heads 2/3.
            fillers = []
            for sbg in range(4):
                fillers.append(gen_v(sbg))            # 4 yields each -> 16 chunks
            fillers.append(gen_proj_qk(wk_sb, bk_sb, kT_sb, 0, 1))
            fillers.append(gen_proj_qk(wq_sb, bq_sb, qT_sb, 0, 1))
            fillers.append(gen_proj_qk(wk_sb, bk_sb, kT_sb, 1, 0))
            fillers.append(gen_proj_qk(wq_sb, bq_sb, qT_sb, 1, 0))
            fillers.append(gen_proj_qk(wk_sb, bk_sb, kT_sb, 1, 1))
            fillers.append(gen_proj_qk(wq_sb, bq_sb, qT_sb, 1, 1))

            units = []
            for h in range(HPC):
                g, hh = divmod(h, 2)
                for half in range(2):
                    for sb in range(NB):
                        units.append((g, hh, half, sb))

            # Pacing: each unit's producers are emitted before it (v[sb]
            # just-in-time; kg0h1 drained before sb=8; qg0h1 before half 1;
            # g1 projections spread over h0-half1/h1 and drained before h2).
            vgens, projgens = fillers[:4], fillers[4:]
            kg0h1, qg0h1 = projgens[0], projgens[1]
            rest = projgens[2:]

            def step(gen):
                try:
                    next(gen)
                    return True
                except StopIteration:
                    return False

            def drain(gen):
                for _ in gen:
                    pass

            pending = list(rest)
            for i, (g, hh, half, sb) in enumerate(units):
                if i < 16:
                    step(vgens[sb // 4])
                    if sb == 8:
                        drain(kg0h1)
                    elif sb < 8:
                        step(kg0h1)
                elif i == 16:
                    drain(qg0h1)
                elif i == 64:
                    for gn in pending:
                        drain(gn)
                    pending = []
                elif pending:
                    if not step(pending[0]):
                        pending.pop(0)
                emit_unit(g, hh, half, sb)
            for gn in pending:
                drain(gn)
            drain(kg0h1)
            drain(qg0h1)

    nc.compile()
    return nc


def _get_nc():
    if not _nc_cache:
        _nc_cache.append(_build_nc())
    return _nc_cache[0]


def _shard_inputs(X, residual_score, wq, wk, wv, bq, bk, bv, bo):
    import ml_dtypes

    bf = ml_dtypes.bfloat16

    def p_major(a):  # [C*128, N] -> [128, C*N] with SBUF partition-major layout
        C = a.shape[0] // 128
        return np.ascontiguousarray(
            a.reshape(C, 128, -1).transpose(1, 0, 2).reshape(128, -1)
        )

    xts = [p_major(X[b].T.astype(bf)) for b in range(B)]
    in_maps = []
    for core in range(N_CORES):
        b, hg = divmod(core, HG)
        c0 = hg * DQ
        cs = slice(c0, c0 + DQ)
        in_maps.append(
            {
                "xt": xts[b],
                "wq": p_major(wq[:, cs].astype(bf)),
                "wk": p_major(wk[:, cs].astype(bf)),
                "wv": p_major(wv[:, cs].astype(bf)),
                "bqk": np.ascontiguousarray(
                    np.concatenate(
                        [bq[cs].reshape(2, 128).T, bk[cs].reshape(2, 128).T], axis=1
                    )
                ),
                "bvb": np.ascontiguousarray(np.broadcast_to(bv[cs], (128, DQ))),
                "rt": np.ascontiguousarray((residual_score[b][:, cs] + bo[cs]).T),
            }
        )
    return in_maps


def kernel(X, residual_score, wq, wk, wv, bq, bk, bv, bo, head_num):
    global LAST_RESULT
    from concourse.bass_utils import run_bass_kernel_spmd

    assert int(head_num) == H
    X = np.asarray(X, dtype=np.float32)
    residual_score = np.asarray(residual_score, dtype=np.float32)
    wq = np.asarray(wq, dtype=np.float32)
    wk = np.asarray(wk, dtype=np.float32)
    wv = np.asarray(wv, dtype=np.float32)
    bq = np.asarray(bq, dtype=np.float32)
    bk = np.asarray(bk, dtype=np.float32)
    bv = np.asarray(bv, dtype=np.float32)
    bo = np.asarray(bo, dtype=np.float32)

    nc = _get_nc()
    in_maps = _shard_inputs(X, residual_score, wq, wk, wv, bq, bk, bv, bo)
    res = run_bass_kernel_spmd(
        nc, in_maps, core_ids=list(range(N_CORES)), trace=TRACE
    )
    LAST_RESULT = res

    out = np.empty((B, S, D), dtype=np.float32)
    for core in range(N_CORES):
        b, hg = divmod(core, HG)
        c0 = hg * DQ
        out[b][:, c0 : c0 + DQ] = res.results[core]["ot"].T
    return (out, out)


# revision 18
# speedup vs baseline: 1.0817x; 1.0473x over previous
"""Multi-head QKV tanh-attention (B=2,S=2048,D=1024,H=16) on 8 TRN2 cores.

Sharding: core = (batch, head_group) on a 2x4 grid. Each core computes the
output column slice out[b, :, hg*256:(hg+1)*256] for its 4 heads; no
cross-core communication (host gathers slices). All transposes are done on
the host: the device receives X^T, column-sliced weights, and
(residual+bo)^T, and produces out^T for its slice.

Per-core device program (all matmuls in float32r at full PE rate):
  qT/kT = W^T X^T (+bias per-partition), v = X W_v (+bias bcast)
  per head, per 128-row key block:
    scoresT = kT_blk^T qT          (PSUM)
    attT    = tanh(scoresT/scale)  (ScalarE, scale folded into activation)
    outT   += v_blk^T attT         (PSUM accumulation over key blocks)
  outT + (residual+bo)^T -> DMA out
"""
import sys

if "/opt/trn_rl_repo" not in sys.path:
    sys.path.insert(0, "/opt/trn_rl_repo")

import numpy as np

B, S, D, H = 2, 2048, 1024, 16
N_CORES = 8
HG = 4                # head groups = cores per batch
HPC = H // HG         # heads per core (4)
HD = D // H           # head dim (64)
DQ = HPC * HD         # per-core output columns (256)
NB = S // 128         # key blocks (16)
KD = D // 128         # contraction chunks (8)
SCALE_INV = float(1.0 / np.sqrt(S / H))

TRACE = False
LAST_RESULT = None

_nc_cache = []


def _enable_ldw_opt():
    """Walrus elides back-to-back LDWEIGHTS with identical APs when
    --enable-ldw-opt=true; concourse hardcodes false. Consecutive matmuls
    here reuse the same stationary operand, so flip the flag."""
    from concourse import bass_utils as bu

    if getattr(bu, "_ldw_opt_patched", False):
        return
    orig = bu.run_command

    def patched(argv, **kwargs):
        argv = [
            a.replace("--enable-ldw-opt=false", "--enable-ldw-opt=true")
            if isinstance(a, str)
            else a
            for a in argv
        ]
        return orig(argv, **kwargs)

    bu.run_command = patched
    bu._ldw_opt_patched = True


def _build_nc():
    from concourse import bacc, mybir
    import concourse.tile as tile


    f32 = mybir.dt.float32
    bf16 = mybir.dt.bfloat16
    Tanh = mybir.ActivationFunctionType.Tanh

    nc = bacc.Bacc(
        "TRN2",
        target_bir_lowering=False,
        debug=False,
        enable_asserts=False,
        num_devices=N_CORES,
    )
    xt_d = nc.dram_tensor("xt", [128, KD * S], bf16, kind="ExternalInput")
    wq_d = nc.dram_tensor("wq", [128, KD * DQ], bf16, kind="ExternalInput")
    wk_d = nc.dram_tensor("wk", [128, KD * DQ], bf16, kind="ExternalInput")
    wv_d = nc.dram_tensor("wv", [128, KD * DQ], bf16, kind="ExternalInput")
    bqk_d = nc.dram_tensor("bqk", [128, 4], f32, kind="ExternalInput")
    bvb_d = nc.dram_tensor("bvb", [128, DQ], f32, kind="ExternalInput")
    rt_d = nc.dram_tensor("rt", [DQ, S], bf16, kind="ExternalInput")
    ot_d = nc.dram_tensor("ot", [DQ, S], f32, kind="ExternalOutput")

    with tile.TileContext(nc) as tc:
        with (
            tc.tile_pool(name="xt", bufs=1) as xt_pool,
            tc.tile_pool(name="w", bufs=3) as w_pool,
            tc.tile_pool(name="qk", bufs=4) as qk_pool,
            tc.tile_pool(name="v", bufs=NB) as v_pool,
            tc.tile_pool(name="rt", bufs=HPC) as rt_pool,
            tc.tile_pool(name="bias", bufs=3) as bias_pool,
            tc.tile_pool(name="att", bufs=6) as att_pool,
            tc.tile_pool(name="osb", bufs=2) as out_pool,
            tc.tile_pool(name="junk", bufs=2) as junk_pool,
            # PSUM: psA [128,1024]x3 = 6 banks, pso [64,1024]x1 = 2 banks
            tc.tile_pool(name="psA", bufs=3, space="PSUM") as psA_pool,
            tc.tile_pool(name="pso", bufs=1, space="PSUM") as pso_pool,
        ):
            # ---- PE warmup: junk matmuls while input DMAs stream ------------
            jw = junk_pool.tile([128, 512], bf16, tag="junk", name="jw")
            nc.vector.memset(jw[:], 0.0)
            jp = psA_pool.tile([128, 1024], f32, tag="psA", name="jp")
            for _ in range(16):
                nc.tensor.matmul(jp[:, 0:512], jw[:, 0:128], jw[:], start=True, stop=True)

            # ---- input DMAs -------------------------------------------------
            # Critical path (wk, wq, xt, wv) on the fast sync HWDGE ring in
            # first-needed order; tiny biases on the scalar ring; rt (needed
            # only at epilogues) on SWDGE so it does not steal HBM bandwidth
            # from the projection inputs.
            w_tiles = {}
            for nm, dram in (("wk", wk_d), ("wq", wq_d)):
                t = w_pool.tile([128, KD * DQ], bf16, tag="w", name=f"{nm}_all")
                nc.sync.dma_start(t[:], dram.ap())
                w_tiles[nm] = t
            xt_all = xt_pool.tile([128, KD * S], bf16, tag="xt", name="xt_all")
            for q in range(4):
                nc.sync.dma_start(
                    xt_all[:, q * 2 * S : (q + 1) * 2 * S],
                    xt_d.ap()[:, q * 2 * S : (q + 1) * 2 * S],
                )
            t = w_pool.tile([128, KD * DQ], bf16, tag="w", name="wv_all")
            nc.sync.dma_start(t[:], wv_d.ap())
            w_tiles["wv"] = t
            bqk_sb = bias_pool.tile([128, 4], f32, tag="b", name="bqks")
            nc.scalar.dma_start(bqk_sb[:], bqk_d.ap())
            bq_sb = bqk_sb[:, 0:2]
            bk_sb = bqk_sb[:, 2:4]
            bvb_sb = bias_pool.tile([128, DQ], f32, tag="bvb", name="bvbs")
            nc.scalar.dma_start(bvb_sb[:], bvb_d.ap())
            rt_sb = []
            for h in range(HPC):
                t = rt_pool.tile([64, S], bf16, tag="rt", name=f"rt{h}")
                nc.gpsimd.dma_start(t[:], rt_d.ap()[h * 64 : (h + 1) * 64, :])
                rt_sb.append(t)

            class _View:
                def __init__(self, base, stride):
                    self.base = base
                    self.stride = stride

                def __getitem__(self, i):
                    class _Sl:
                        def __init__(so, base, off):
                            so.base = base
                            so.off = off

                        def __getitem__(so, key):
                            rows, cols = key
                            return so.base[rows, so.off + cols.start : so.off + cols.stop]

                    return _Sl(self.base, i * self.stride)

            xt_sb = _View(xt_all, S)
            wk_sb = _View(w_tiles["wk"], DQ)
            wq_sb = _View(w_tiles["wq"], DQ)
            wv_sb = _View(w_tiles["wv"], DQ)

            qT_sb = [qk_pool.tile([128, S], bf16, tag="qk", name=f"qT{g}") for g in range(2)]
            kT_sb = [qk_pool.tile([128, S], bf16, tag="qk", name=f"kT{g}") for g in range(2)]
            v_sb = [v_pool.tile([128, DQ], bf16, tag="v", name=f"v{sb}") for sb in range(NB)]

            # ---- emission helpers -------------------------------------------
            # Generators yield after small PE chunks so attention units can be
            # interleaved between them (keeps ScalarE fed with tanh work).
            def gen_proj_qk(w_sb, b_sb, dst, g, half):
                hsl = slice(half * 1024, (half + 1) * 1024)
                ps = psA_pool.tile([128, 1024], f32, tag="psA", name="psp")
                for din in range(KD):
                    for ns in range(2):
                        nc.tensor.matmul(
                            ps[:, ns * 512 : (ns + 1) * 512],
                            w_sb[din][:, g * 128 : (g + 1) * 128],
                            xt_sb[din][:, half * 1024 + ns * 512 :
                                       half * 1024 + (ns + 1) * 512],
                            start=(din == 0),
                            stop=(din == KD - 1),
                            skip_group_check=True,
                        )
                    if din < KD - 1:
                        yield
                nc.vector.tensor_scalar_add(dst[g][:, hsl], ps[:], b_sb[:, g : g + 1])

            def gen_v(sbg):
                ps = psA_pool.tile([128, 1024], f32, tag="psA", name="psv")
                for sbi in range(4):
                    sb = sbg * 4 + sbi
                    for din in range(KD):
                        nc.tensor.matmul(
                            ps[:, sbi * DQ : (sbi + 1) * DQ],
                            xt_sb[din][:, sb * 128 : (sb + 1) * 128],
                            wv_sb[din][:, 0:DQ],
                            start=(din == 0),
                            stop=(din == KD - 1),
                            skip_group_check=True,
                        )
                    nc.vector.tensor_add(
                        v_sb[sb][:], ps[:, sbi * DQ : (sbi + 1) * DQ], bvb_sb[:]
                    )
                    if sbi < 3:
                        yield

            def run_gen(gen):
                for _ in gen:
                    pass

            o_tiles = {}
            pso_tiles = {}

            def emit_unit(g, hh, half, sb):
                # one attention unit: scoresT -> tanh -> AV for one key block
                h = 2 * g + hh
                po = hh * 64
                key = (h, half)
                if key not in pso_tiles:
                    pso_tiles[key] = pso_pool.tile([64, 1024], f32, tag="pso", name="ps_o")
                ps_o = pso_tiles[key]
                ps_s = psA_pool.tile([128, 1024], f32, tag="psA", name="pss")
                for ns in range(2):
                    nc.tensor.matmul(
                        ps_s[:, ns * 512 : (ns + 1) * 512],
                        kT_sb[g][po : po + 64, sb * 128 : (sb + 1) * 128],
                        qT_sb[g][po : po + 64, half * 1024 + ns * 512 :
                                 half * 1024 + (ns + 1) * 512],
                        start=True,
                        stop=True,
                        skip_group_check=True,
                    )
                att = att_pool.tile([128, 1024], bf16, tag="att", name="att")
                nc.scalar.activation(att[:], ps_s[:], Tanh, scale=SCALE_INV)
                for ns in range(2):
                    nc.tensor.matmul(
                        ps_o[:, ns * 512 : (ns + 1) * 512],
                        v_sb[sb][:, h * HD : (h + 1) * HD],
                        att[:, ns * 512 : (ns + 1) * 512],
                        start=(sb == 0),
                        stop=(sb == NB - 1),
                        skip_group_check=True,
                    )
                if sb == NB - 1:
                    if h not in o_tiles:
                        o_tiles[h] = out_pool.tile([64, S], f32, tag="osb", name=f"o{h}")
                    o_sb = o_tiles[h]
                    hsl = slice(half * 1024, (half + 1) * 1024)
                    nc.vector.tensor_add(o_sb[:, hsl], ps_o[:], rt_sb[h][:, hsl])
                    nc.sync.dma_start(
                        ot_d.ap()[h * 64 : (h + 1) * 64,
                                  half * 1024 : (half + 1) * 1024],
                        o_sb[:, hsl],
                    )
                    del pso_tiles[key]
                    if half == 1:
                        del o_tiles[h]

            # ---- schedule ---------------------------------------------------
            # Minimal prefix before attention can start: kT/qT of group 0,
            # s-half 0 (covers key blocks 0-7 for heads 0/1).
            _gk = gen_proj_qk(wk_sb, bk_sb, kT_sb, 0, 0)
            _gq = gen_proj_qk(wq_sb, bq_sb, qT_sb, 0, 0)
            _alive = True
            while _alive:
                _alive = False
                for _g in (_gk, _gq):
                    try:
                        next(_g)
                        _alive = True
                    except StopIteration:
                        pass

            # Fillers, ordered by when their results are first needed:
            #   v[sb] before unit (h0, half0, sb); kg0h1 before sb>=8 of h0;
            #   qg0h1 before h0 half1; g1 projections before heads 2/3.
            fillers = []
            for sbg in range(4):
                fillers.append(gen_v(sbg))            # 4 yields each -> 16 chunks
            fillers.append(gen_proj_qk(wk_sb, bk_sb, kT_sb, 0, 1))
            fillers.append(gen_proj_qk(wq_sb, bq_sb, qT_sb, 0, 1))
            fillers.append(gen_proj_qk(wk_sb, bk_sb, kT_sb, 1, 0))
            fillers.append(gen_proj_qk(wq_sb, bq_sb, qT_sb, 1, 0))
            fillers.append(gen_proj_qk(wk_sb, bk_sb, kT_sb, 1, 1))
            fillers.append(gen_proj_qk(wq_sb, bq_sb, qT_sb, 1, 1))

            units = []
            for h in range(HPC):
                g, hh = divmod(h, 2)
                for half in range(2):
                    for sb in range(NB):
                        units.append((g, hh, half, sb))

            # Pacing: each unit's producers are emitted before it (v[sb]
            # just-in-time; kg0h1 drained before sb=8; qg0h1 before half 1;
            # g1 projections spread over h0-half1/h1 and drained before h2).
            vgens, projgens = fillers[:4], fillers[4:]
            kg0h1, qg0h1 = projgens[0], projgens[1]
            rest = projgens[2:]

            def step(gen):
                try:
                    next(gen)
                    return True
                except StopIteration:
                    return False

            def drain(gen):
                for _ in gen:
                    pass

            pending = list(rest)
            for i, (g, hh, half, sb) in enumerate(units):
                if i < 16:
                    step(vgens[sb // 4])
                    if sb < 8:
                        step(kg0h1)
                        if sb == 7:
                            drain(kg0h1)
                    else:
                        step(qg0h1)
                        if sb == 15:
                            drain(qg0h1)
                elif i == 64:
                    for gn in pending:
                        drain(gn)
                    pending = []
                elif pending:
                    if not step(pending[0]):
                        pending.pop(0)
                emit_unit(g, hh, half, sb)
            for gn in pending:
                drain(gn)

    nc.compile()
    return nc


def _get_nc():
    if not _nc_cache:
        _nc_cache.append(_build_nc())
    return _nc_cache[0]


def _shard_inputs(X, residual_score, wq, wk, wv, bq, bk, bv, bo):
    import ml_dtypes

    bf = ml_dtypes.bfloat16

    def p_major(a):  # [C*128, N] -> [128, C*N] with SBUF partition-major layout
        C = a.shape[0] // 128
        return np.ascontiguousarray(
            a.reshape(C, 128, -1).transpose(1, 0, 2).reshape(128, -1)
        )

    xts = [p_major(X[b].T.astype(bf)) for b in range(B)]
    in_maps = []
    for core in range(N_CORES):
        b, hg = divmod(core, HG)
        c0 = hg * DQ
        cs = slice(c0, c0 + DQ)
        in_maps.append(
            {
                "xt": xts[b],
                "wq": p_major(wq[:, cs].astype(bf)),
                "wk": p_major(wk[:, cs].astype(bf)),
                "wv": p_major(wv[:, cs].astype(bf)),
                "bqk": np.ascontiguousarray(
                    np.concatenate(
                        [bq[cs].reshape(2, 128).T, bk[cs].reshape(2, 128).T], axis=1
                    )
                ),
                "bvb": np.ascontiguousarray(np.broadcast_to(bv[cs], (128, DQ))),
                "rt": np.ascontiguousarray((residual_score[b][:, cs] + bo[cs]).T).astype(bf),
            }
        )
    return in_maps


def kernel(X, residual_score, wq, wk, wv, bq, bk, bv, bo, head_num):
    global LAST_RESULT
    from concourse.bass_utils import run_bass_kernel_spmd

    assert int(head_num) == H
    X = np.asarray(X, dtype=np.float32)
    residual_score = np.asarray(residual_score, dtype=np.float32)
    wq = np.asarray(wq, dtype=np.float32)
    wk = np.asarray(wk, dtype=np.float32)
    wv = np.asarray(wv, dtype=np.float32)
    bq = np.asarray(bq, dtype=np.float32)
    bk = np.asarray(bk, dtype=np.float32)
    bv = np.asarray(bv, dtype=np.float32)
    bo = np.asarray(bo, dtype=np.float32)

    nc = _get_nc()
    in_maps = _shard_inputs(X, residual_score, wq, wk, wv, bq, bk, bv, bo)
    res = run_bass_kernel_spmd(
        nc, in_maps, core_ids=list(range(N_CORES)), trace=TRACE
    )
    LAST_RESULT = res

    out = np.empty((B, S, D), dtype=np.float32)
    for core in range(N_CORES):
        b, hg = divmod(core, HG)
        c0 = hg * DQ
        out[b][:, c0 : c0 + DQ] = res.results[core]["ot"].T
    return (out, out)
